# revision 46
# baseline (speedup 1.0000x reference)
"""ESMFold gated attention (B=8, Q=K=1024, C=256, H=8, DH=32) on 8 TRN2 NeuronCores.

Sharding: batch data-parallel, ZERO collectives. Core c owns batch c
end-to-end: projections for all 8 heads, per-head attention, gating,
and the output projection. The host stacks the 8 per-core [Q, C] outputs.
The price vs. head-parallel is that every core streams the full
bias_pair (16 MB bf16) from its DRAM; the win is dropping the AllToAll,
whose fixed cost dominated the previous design (~1.4 ms of the 1.67 ms).

Device pipeline per core (layouts transposed host-side; all matmuls bf16
with fp32 PSUM accumulation):
  1. Projections, all 8 heads at once: qT/gT/kT [256, Q] via two 128-row
     output chunks each (Wq pre-scaled by 1/sqrt(DH) host-side; sigmoid
     gate fused with bg via ACT per-partition bias); v natural [k, dh]
     stored per-head with a ones column appended ([128, KT, H, 33]) so
     row 32 of the AV accumulator yields softmax denominators for free.
  2. Attention per head h, software-pipelined over k-tiles j:
     scores^T[k,q] = kT_h.T @ qT_h into PSUM; bias_pair tile (streamed
     bf16, double-buffered per head) added by a per-j engine schedule
     (DVE / Pool / PE-identity-matmul accumulate) to keep every engine
     under the ACT exp roofline; exp folds bias_mask via ACT per-partition
     bias; AV(j) emitted after scores(j+1) so PE overlaps the add/exp chain.
  3. Per-head epilogue: evict o rows to a 4-head stacked tile (Pool),
     reciprocal of denominators straight from PSUM row 32 (DVE). Per
     4-head group: indicator-matmul broadcast of the 4 reciprocals,
     gate+normalize multiplies -> ogT chunk [128, Q].
  4. Output projection in natural [seq, C] layout (+bo), DMA out.
"""

import math

import numpy as np

import concourse.bass as bass
import concourse.mybir as mybir
import concourse.tile as tile

F32 = mybir.dt.float32
BF16 = mybir.dt.bfloat16

B, Q, K, C, H, DH = 8, 1024, 1024, 256, 8, 32
N_CORES = 8
KT = K // 128  # 8 k-tiles
INV_SQRT_DH = 1.0 / math.sqrt(DH)

# Per-j engine for the bias_pair add: 'v' = DVE tensor_add, 'p' = Pool
# tensor_add, 'P' = PE identity-matmul accumulated into the scores PSUM.
ADD_SCHED_EVEN = ["P", "v", "P", "v", "P", "v", "P", "v"]
ADD_SCHED_ODD = ["P", "v", "v", "P", "v", "P", "v", "v"]
# j at which the previous head's deferred evict+reciprocal are emitted
TAIL_J = 4
# how many (h, j) steps the AV matmuls lag the scores/add/exp front
# (deep lag keeps the in-order PE queue from ever waiting on an exp,
# which also keeps the tensor engine continuously busy for p-state)
AV_LAG = 4


def _split_multi_waits(nc):
    """The walrus build here allows at most one sem wait per instruction
    ("Too many sync wait commands"); move extra waits onto NoOps inserted
    just before, on the same engine (sequencers execute in order)."""
    ctr = 0
    for fn in nc.m.functions:
        for blk in fn.blocks:
            il = blk.instructions
            if not any(
                i.sync_info and i.sync_info.on_wait and len(i.sync_info.on_wait) > 1
                for i in il
            ):
                continue
            out = []
            for inst in il:
                si = inst.sync_info
                if si and si.on_wait and len(si.on_wait) > 1:
                    waits = list(si.on_wait)
                    for w in waits[:-1]:
                        ctr += 1
                        nop = mybir.InstNoOp(name=f"waitnop-{ctr}", ins=[], outs=[])
                        nop.engine = inst.engine
                        nop.sync_info = mybir.SyncInfo(on_wait=[w], on_update=[])
                        out.append(nop)
                    inst.sync_info = mybir.SyncInfo(
                        on_wait=[waits[-1]], on_update=list(si.on_update)
                    )
                out.append(inst)
            blk.instructions = out


def build_kernel(repeat: int = 1, timing_internal_inputs: bool = False) -> bass.Bass:
    nc = bass.Bass("TRN2", target_bir_lowering=False, debug=False, num_devices=N_CORES)

    # ---- per-core inputs (host pre-sharded / pre-transposed) ----
    if timing_internal_inputs:
        # timing-only variant: inputs live in (uninitialized) internal DRAM so
        # per-exec host staging doesn't pollute the measurement
        nc.declare_dram_parameter("tin", [128, 4], F32, isOutput=False)
        xqT = nc.dram_tensor("t_xqT", [C, Q], BF16)
        xkvT = nc.dram_tensor("t_xkvT", [C, K], BF16)
        maskT = nc.dram_tensor("t_maskT", [128, KT], F32)
        biasT = nc.dram_tensor("t_biasT", [H, 128, KT, Q], BF16)
        wq = nc.dram_tensor("t_wq", [2, 128, C], BF16)
        wg = nc.dram_tensor("t_wg", [2, 128, C], BF16)
        wk = nc.dram_tensor("t_wk", [2, 128, C], BF16)
        wv = nc.dram_tensor("t_wv", [2, 128, C], BF16)
        bgr = nc.dram_tensor("t_bgr", [128, 2], F32)
        wo = nc.dram_tensor("t_wo", [2, 128, C], BF16)
        bor = nc.dram_tensor("t_bor", [128, C], F32)
        ident = nc.dram_tensor("t_ident", [128, 128], BF16)
        indq = nc.dram_tensor("t_indq", [128, 128], BF16)
    else:
        xqT = nc.declare_dram_parameter("xqT", [C, Q], BF16, isOutput=False)
        xkvT = nc.declare_dram_parameter("xkvT", [C, K], BF16, isOutput=False)
        maskT = nc.declare_dram_parameter("maskT", [128, KT], F32, isOutput=False)
        biasT = nc.declare_dram_parameter("biasT", [H, 128, KT, Q], BF16, isOutput=False)
        wq = nc.declare_dram_parameter("wq", [2, 128, C], BF16, isOutput=False)
        wg = nc.declare_dram_parameter("wg", [2, 128, C], BF16, isOutput=False)
        wk = nc.declare_dram_parameter("wk", [2, 128, C], BF16, isOutput=False)
        wv = nc.declare_dram_parameter("wv", [2, 128, C], BF16, isOutput=False)
        bgr = nc.declare_dram_parameter("bgr", [128, 2], F32, isOutput=False)
        wo = nc.declare_dram_parameter("wo", [2, 128, C], BF16, isOutput=False)
        bor = nc.declare_dram_parameter("bor", [128, C], F32, isOutput=False)
        ident = nc.declare_dram_parameter("ident", [128, 128], BF16, isOutput=False)
        indq = nc.declare_dram_parameter("indq", [128, 128], BF16, isOutput=False)
    out = nc.declare_dram_parameter("out", [Q, C], F32, isOutput=True)

    with tile.TileContext(nc) as tc:
        with (
            tc.tile_pool(name="const", bufs=1) as const,
            tc.tile_pool(name="bias", bufs=3) as biasp,
            tc.tile_pool(name="xin", bufs=3) as xin,
            tc.tile_pool(name="proj", bufs=2) as proj,
            tc.tile_pool(name="stp", bufs=4) as stp,
            tc.tile_pool(name="atp", bufs=AV_LAG + 2) as atp,
            tc.tile_pool(name="epi", bufs=2) as epi,
            tc.tile_pool(name="ogp", bufs=2) as ogp,
            tc.tile_pool(name="fin", bufs=2) as finp,
            # PSUM budget (8 banks of 2KB/partition): "s" slots 3x2 banks
            # (scores; also borrowed by projections, the r-broadcast, and the
            # final output projection) | "o" 1x2 banks: ONE [128, Q] tile
            # shared by alternating heads (rows 0:33 / 64:97) so head h's
            # evict+reciprocal have a full head period of slack before
            # head h+2 reuses its rows
            tc.tile_pool(name="ps_s", bufs=3, space="PSUM") as ps_s,
            tc.tile_pool(name="ps_o", bufs=1, space="PSUM") as ps_o,
        ):
            # ---- constants ----
            wq_sb = const.tile([128, 2, C], BF16)
            nc.sync.dma_start(wq_sb[:], wq.rearrange("t p m -> p t m"))
            wg_sb = const.tile([128, 2, C], BF16)
            nc.sync.dma_start(wg_sb[:], wg.rearrange("t p m -> p t m"))
            wk_sb = const.tile([128, 2, C], BF16)
            nc.sync.dma_start(wk_sb[:], wk.rearrange("t p m -> p t m"))
            wv_sb = const.tile([128, 2, C], BF16)
            nc.sync.dma_start(wv_sb[:], wv.rearrange("t p m -> p t m"))
            bg_sb = const.tile([128, 2], F32)
            nc.sync.dma_start(bg_sb[:], bgr[:])
            wo_sb = const.tile([128, 2, C], BF16)
            nc.sync.dma_start(wo_sb[:], wo.rearrange("t p m -> p t m"))
            bo_sb = const.tile([128, C], F32)
            nc.sync.dma_start(bo_sb[:], bor[:])
            mask_sb = const.tile([128, KT], F32)
            nc.sync.dma_start(mask_sb[:], maskT[:])
            id_sb = const.tile([128, 128], BF16)
            nc.sync.dma_start(id_sb[:], ident[:])
            # block indicator for the reciprocal broadcast: ind[32c, p] = 1
            # iff p // 32 == c (host-built). The reciprocals live on rows
            # {0,32,64,96} of persistent r4 tiles (engine partition bases
            # must be 32-aligned); junk rows are zeroed once so the zero
            # indicator weights cannot turn garbage NaNs into output NaNs.
            ind_sb = const.tile([128, 128], BF16)
            nc.sync.dma_start(ind_sb[:], indq[:])
            r4c = [
                const.tile([128, Q], BF16, name=f"r4c_{gg}") for gg in range(2)
            ]
            nc.vector.memset(r4c[0][:], 0.0)
            nc.vector.memset(r4c[1][:], 0.0)

            for _rep in range(repeat):
                # per-head bias_pair tiles are DMA'd into biasp inside the
                # head loop (double-buffered, prefetched one head ahead)
                xq_sb = xin.tile([128, 2, Q], BF16, tag="xq")
                nc.sync.dma_start(xq_sb[:], xqT.rearrange("(t p) q -> p t q", p=128))
                xkv_sb = xin.tile([128, 2, K], BF16, tag="xkv")
                nc.sync.dma_start(xkv_sb[:], xkvT.rearrange("(t p) q -> p t q", p=128))

                # ---- projections: all 8 heads up front ----
                # qT/kT as [64, 4, Q] (matmul operand slices must start at
                # partition 0/32/64, so head h sits at partition 32*(h%2) of
                # chunk h//2); gT is only ever a DVE operand so it keeps the
                # denser [128, 2, Q] layout; v as [128, KT, H, 33]
                q_sb = proj.tile([64, 4, Q], BF16, tag="q")
                g_sb = proj.tile([128, 2, Q], BF16, tag="g")
                k_sb = proj.tile([64, 4, K], BF16, tag="k")
                v_sb = proj.tile([128, KT, H, DH + 1], BF16, tag="v")
                nc.gpsimd.memset(v_sb[:, :, :, DH : DH + 1], 1.0)

                def emit_mat(w_sb, dst, evict, xsrc, nco):
                    wd = 256 // nco
                    for co in range(nco):
                        ps = ps_s.tile([wd, Q], F32, tag="s", name=f"ps_{wd}")
                        for ch in range(2):
                            for ct in range(2):
                                nc.tensor.matmul(
                                    ps[:, ch * 512 : (ch + 1) * 512],
                                    lhsT=w_sb[:, ct, co * wd : (co + 1) * wd],
                                    rhs=xsrc[:, ct, ch * 512 : (ch + 1) * 512],
                                    start=(ct == 0),
                                    stop=(ct == 1),
                                )
                        evict(co, dst, ps)

                def ev_copy_dve(co, dst, ps):
                    nc.vector.tensor_copy(dst[:, co, :], ps[:])

                def ev_copy_act(co, dst, ps):
                    nc.scalar.activation(
                        dst[:, co, :], ps[:], mybir.ActivationFunctionType.Copy
                    )

                def ev_sigmoid(co, dst, ps):
                    nc.scalar.activation(
                        dst[:, co, :],
                        ps[:],
                        mybir.ActivationFunctionType.Sigmoid,
                        bias=bg_sb[:, co : co + 1],
                    )

                emit_mat(wq_sb, q_sb, ev_copy_dve, xq_sb, 4)
                emit_mat(wg_sb, g_sb, ev_sigmoid, xq_sb, 2)
                emit_mat(wk_sb, k_sb, ev_copy_act, xkv_sb, 4)
                # v: per k-tile j, [128, H*DH] -> strided into the 33-col
                # per-head layout
                for j in range(KT):
                    v_ps = ps_s.tile([128, H, DH], F32, tag="s")
                    for ct in range(2):
                        nc.tensor.matmul(
                            v_ps[:],
                            lhsT=xkv_sb[:, ct, j * 128 : (j + 1) * 128],
                            rhs=wv_sb[:, ct, :],
                            start=(ct == 0),
                            stop=(ct == 1),
                        )
                    nc.vector.tensor_copy(v_sb[:, j, :, 0:DH], v_ps[:])

                og_sb = ogp.tile([128, 2, Q], BF16, tag="og")
                o4_sb = [
                    epi.tile([128, Q], F32, tag=f"o4_{gg}", name=f"o4_{gg}")
                    for gg in range(2)
                ]

                bias_tiles = {}

                def fetch_bias(h):
                    eb = biasp.tile([128, KT, Q], BF16, tag="eb")
                    nc.sync.dma_start(eb[:], biasT[h])
                    bias_tiles[h] = eb

                def group_epilogue(gg):
                    # 4 heads of group gg are in o4_sb[gg] (rows h%4*32..)
                    # and their denominators' reciprocals on rows {0,32,64,96}
                    # of r4c[gg]. Broadcast across each 32-row band via the
                    # block-indicator matmul, then gate+normalize into ogT
                    # chunk gg.
                    gr_sb = epi.tile([128, Q], F32, tag="gr")
                    for ch in range(2):
                        rb_ps = ps_s.tile([128, Q // 2], F32, tag="s")
                        nc.tensor.matmul(
                            rb_ps[:],
                            lhsT=ind_sb[:],
                            rhs=r4c[gg][:, ch * 512 : (ch + 1) * 512],
                            start=True,
                            stop=True,
                        )
                        nc.vector.tensor_mul(
                            gr_sb[:, ch * 512 : (ch + 1) * 512],
                            g_sb[:, gg, ch * 512 : (ch + 1) * 512],
                            rb_ps[:],
                        )
                    nc.gpsimd.tensor_mul(og_sb[:, gg, :], o4_sb[gg][:], gr_sb[:])

                # one [128, Q] PSUM accumulator shared by alternating heads:
                # head h accumulates o_aug in rows op..op+32 with op=64*(h%2)
                o_pair = ps_o.tile([128, Q], F32, tag="o")

                def emit_tail(h):
                    # deferred evict + reciprocal for head h (emitted inside
                    # head h+1's j-loop, safely before head h+2 reuses the
                    # rows): o rows into the stacked 4-head tile (Pool),
                    # denominator reciprocal straight from PSUM (DVE)
                    op = 64 * (h % 2)
                    gg = h // 4
                    hp4 = 32 * (h % 4)
                    nc.vector.tensor_copy(
                        o4_sb[gg][hp4 : hp4 + 32, :], o_pair[op : op + DH, :]
                    )
                    with nc.allow_low_precision(
                        reason="bf16 reciprocal of softmax denominator; "
                        "0.4% on a common normalization factor"
                    ):
                        nc.vector.reciprocal(
                            r4c[gg][hp4 : hp4 + 1, :],
                            o_pair[op + DH : op + DH + 1, :],
                        )

                def _av(h2, jj, a):
                    op2 = 64 * (h2 % 2)
                    for ch in range(2):
                        nc.tensor.matmul(
                            o_pair[op2 : op2 + DH + 1, ch * 512 : (ch + 1) * 512],
                            lhsT=v_sb[:, jj, h2, :],
                            rhs=a[:, ch * 512 : (ch + 1) * 512],
                            start=(jj == 0),
                            stop=(jj == KT - 1),
                        )

                # one flat software-pipelined stream over (h, j): AV lags the
                # scores/add/exp front by AV_LAG steps and crosses head
                # boundaries, so the next head's scores issue before the
                # previous head's last AVs and the in-order PE queue never
                # drains waiting on the final exp. Previous head's
                # evict+reciprocal land at j == TAIL_J, group-0's epilogue at
                # (h=5, j=3).
                fetch_bias(0)
                fetch_bias(1)
                fetch_bias(2)
                seq = [(h, j) for h in range(H) for j in range(KT)]
                av_pend = []
                for t, (h, j) in enumerate(seq):
                    hp = 32 * (h % 2)
                    co = h // 2
                    if j == 0 and h + 3 < H:
                        fetch_bias(h + 3)
                    eng = (ADD_SCHED_EVEN if h % 2 == 0 else ADD_SCHED_ODD)[j]
                    eb = bias_tiles[h]
                    s_ps = ps_s.tile([128, Q], F32, tag="s")
                    for ch in range(2):
                        nc.tensor.matmul(
                            s_ps[:, ch * 512 : (ch + 1) * 512],
                            lhsT=k_sb[hp : hp + 32, co, j * 128 : (j + 1) * 128],
                            rhs=q_sb[hp : hp + 32, co, ch * 512 : (ch + 1) * 512],
                            start=True,
                            stop=(eng != "P"),
                        )
                        if eng == "P":
                            nc.tensor.matmul(
                                s_ps[:, ch * 512 : (ch + 1) * 512],
                                lhsT=id_sb[:],
                                rhs=eb[:, j, ch * 512 : (ch + 1) * 512],
                                start=False,
                                stop=True,
                            )
                    if eng == "P":
                        esrc = s_ps
                    else:
                        st = stp.tile([128, Q], F32, tag="st")
                        if eng == "v":
                            nc.vector.tensor_add(st[:], s_ps[:], eb[:, j, :])
                        else:
                            nc.gpsimd.tensor_add(st[:], s_ps[:], eb[:, j, :])
                        esrc = st
                    at = atp.tile([128, Q], BF16, tag="at")
                    nc.scalar.activation(
                        at[:],
                        esrc[:],
                        mybir.ActivationFunctionType.Exp,
                        bias=mask_sb[:, j : j + 1],
                    )
                    av_pend.append((h, j, at))
                    if len(av_pend) > AV_LAG:
                        _av(*av_pend.pop(0))
                    if j == 3 and h == 5:
                        group_epilogue(0)
                    if j == TAIL_J and h > 0:
                        emit_tail(h - 1)
                for item in av_pend:
                    _av(*item)
                emit_tail(H - 1)
                group_epilogue(1)

                # ---- output projection, natural [seq, C] layout ----
                for si in range(Q // 128):
                    out_ps = ps_s.tile([128, C], F32, tag="s")
                    for ct in range(2):
                        nc.tensor.matmul(
                            out_ps[:],
                            lhsT=og_sb[:, ct, si * 128 : (si + 1) * 128],
                            rhs=wo_sb[:, ct, :],
                            start=(ct == 0),
                            stop=(ct == 1),
                        )
                    out_sb = finp.tile([128, C], F32, tag="outsb")
                    nc.vector.tensor_add(out_sb[:], out_ps[:], bo_sb[:])
                    nc.sync.dma_start(out[si * 128 : (si + 1) * 128, :], out_sb[:])

    _split_multi_waits(nc)
    return nc


def shard_inputs(q_x, kv_x, bias_mask, bias_pair, Wq, Wk, Wv, Wg, bg, Wo, bo):
    """Build the per-core input maps (host-side slicing/layout only)."""
    import ml_dtypes

    bf16 = ml_dtypes.bfloat16
    q_x = np.asarray(q_x, np.float32)
    kv_x = np.asarray(kv_x, np.float32)
    bias_mask = np.asarray(bias_mask, np.float32)
    bias_pair = np.asarray(bias_pair, np.float32)

    # bias_pair^T per head, tiled for partition-linear DMA:
    # biasT[h, p, j, q] = bias_pair[0, h, q, j*128 + p]
    biasT = np.ascontiguousarray(
        bias_pair[0].transpose(0, 2, 1).reshape(H, KT, 128, Q).transpose(0, 2, 1, 3)
    ).astype(bf16)

    wq = np.ascontiguousarray(
        (np.asarray(Wq, np.float32) * INV_SQRT_DH).reshape(2, 128, C)
    ).astype(bf16)
    wg = np.ascontiguousarray(np.asarray(Wg, np.float32).reshape(2, 128, C)).astype(bf16)
    wk = np.ascontiguousarray(np.asarray(Wk, np.float32).reshape(2, 128, C)).astype(bf16)
    wv = np.ascontiguousarray(np.asarray(Wv, np.float32).reshape(2, 128, C)).astype(bf16)
    bgr = np.ascontiguousarray(np.asarray(bg, np.float32).reshape(2, 128).T)
    wo = np.ascontiguousarray(np.asarray(Wo, np.float32).reshape(2, 128, C)).astype(bf16)
    bor = np.ascontiguousarray(
        np.broadcast_to(np.asarray(bo, np.float32), (128, C))
    )
    ident = np.eye(128, dtype=bf16)
    indq = np.zeros((128, 128), dtype=bf16)
    for c4 in range(4):
        indq[c4 * 32, c4 * 32 : (c4 + 1) * 32] = 1

    in_maps = []
    for c in range(N_CORES):
        in_maps.append(
            {
                "xqT": np.ascontiguousarray(q_x[c].T).astype(bf16),
                "xkvT": np.ascontiguousarray(kv_x[c].T).astype(bf16),
                "maskT": np.ascontiguousarray(
                    bias_mask[c, 0, 0].reshape(KT, 128).T
                ),
                "biasT": biasT,
                "wq": wq,
                "wg": wg,
                "wk": wk,
                "wv": wv,
                "bgr": bgr,
                "wo": wo,
                "bor": bor,
                "ident": ident,
                "indq": indq,
            }
        )
    return in_maps


def assemble_output(results):
    out = np.empty((B, Q, C), np.float32)
    for c in range(N_CORES):
        out[c] = results[c]["out"]
    return out


_NC_CACHE = None


def kernel(**inputs) -> np.ndarray:
    global _NC_CACHE
    from concourse.bass_utils import run_bass_kernel_spmd

    if _NC_CACHE is None:
        _NC_CACHE = build_kernel()
    in_maps = shard_inputs(**inputs)
    res = run_bass_kernel_spmd(_NC_CACHE, in_maps, list(range(N_CORES)))
    return assemble_output(res.results)
